# revision 33
# baseline (speedup 1.0000x reference)
"""Trainium2 Bass kernel for blocked-DCT high-frequency extractor.

Computes, for x (64, 3, 512, 512) f32:
  gray = 0.299*R + 0.587*G + 0.114*B                     (B,1,H,W)
  per 8x8 block:  Y = mask * (D @ block @ D.T)           (2D DCT + high-pass)
  output (64, 1, 512, 512) f32

Strategy (pure data parallel over batch, 8 batches/core on 8 cores; the
kernel is HBM-bound: 24 MiB in + 8 MiB out per core, ~358 GB/s/core HBM
=> ~94 us floor; every compute engine is kept well under that wall).

Per core, per (batch, 128-row chunk) of the image:
  1. One 768 KB SWDGE DMA (gpsimd queue) brings all 3 channel chunks into
     a (128h, 3*512w) tile, CASTING fp32 -> bf16 in the DMA datapath.
  2. Grayscale in bf16, split DVE/ACT: g0 = x0*(w0/w2) + x2 (DVE STT),
     gs = x1*(w1/w2) (ACT), g1 = g0 + gs (ACT).
  3. H-direction DCT: one bf16 matmul.  The stationary weight is
     w2 * (I_16 kron D^T) with COLUMNS PERMUTED so the output row index
     is i' = u*16 + hb (u = h-frequency, hb = block row).  After the
     32x32 block transpose this puts u//2 into the partition-block index
     of the next matmul's output.
  4. ACT copies PSUM -> SBUF with fp32 -> bf16 cast (ISA forbids casts
     inside StreamTranspose), then DVE does the 32x32 block transpose.
  5. W-direction DCT: one bf16 matmul whose stationary weight is
     I_16 kron D^T with the high-pass mask FOLDED IN: with the permuted
     layout, mask==0 exactly on output partitions {l < 64 and l%8 < 4},
     so those weight columns are simply zeroed.  No mask stage at all.
  6. DVE 32x32 block transpose straight out of PSUM (fp32).
  7. 256 KB output DMA on the SP HWDGE queue whose DRAM access pattern
     un-permutes the rows: partition c2*32+c1*16+hb -> row hb*8+c2*2+c1.

The two matmuls are bf16 (the PE idles most of the kernel, so HAM holds
it at the cold 1.2 GHz clock; fp32 matmuls at 4x cycles were the
original bottleneck).  All intermediate precision is bf16, fine for the
2e-2 relative-error gate (measured ~6e-3).
"""

import os

# The ASAP tile scheduler interleaves the per-iteration work properly;
# the legacy scheduler phase-separates the MM1/cast work from the
# t1/MM2/t2/store tails, serializing the two halves of the pipeline.
os.environ.setdefault("TILE_SCHEDULER", "asap")

import numpy as np

import concourse.bacc as bacc
import concourse.mybir as mybir
import concourse.tile as tile
from concourse.bass_utils import run_bass_kernel_spmd

N_CORES = 8
B, C, H, W = 64, 3, 512, 512
BLOC = B // N_CORES  # batches per core
P = 128              # SBUF partitions / chunk height
NCH = H // P         # 128-row chunks per image
F32 = mybir.dt.float32
BF16 = mybir.dt.bfloat16
GRAY_W = (0.299, 0.587, 0.114)

_NC = None          # cached compiled Bass module
LAST_RUN = None     # BassKernelResults of the most recent run (for test.py)


def _build_bass():
    nc = bacc.Bacc(
        "TRN2",
        target_bir_lowering=False,
        debug=False,
        num_devices=N_CORES,
    )
    x = nc.declare_dram_parameter("x", [BLOC, C, H, W], F32, isOutput=False)
    wts1 = nc.declare_dram_parameter("wts1", [1, P, P], BF16, isOutput=False)
    wts2 = nc.declare_dram_parameter("wts2", [1, P, P], BF16, isOutput=False)
    out = nc.declare_dram_parameter("out", [BLOC, 1, H, W], F32, isOutput=True)

    ga = GRAY_W[0] / GRAY_W[2]
    gb = GRAY_W[1] / GRAY_W[2]
    mult = mybir.AluOpType.mult
    add = mybir.AluOpType.add

    with tile.TileContext(nc) as tc:
        with (
            tc.tile_pool(name="consts", bufs=1) as consts,
            tc.tile_pool(name="xin", bufs=10) as xin,
            tc.tile_pool(name="work", bufs=8) as work,
            tc.tile_pool(name="psum", bufs=4, space="PSUM") as psum_pool,
        ):
            w1 = consts.tile([P, P], BF16, tag="w1")
            nc.sync.dma_start(w1[:], wts1[0])
            w2 = consts.tile([P, P], BF16, tag="w2")
            nc.sync.dma_start(w2[:], wts2[0])

            iters = []
            for b in range(BLOC):
                for hc in range(NCH):
                    iters.append((b, hc))

            # Software-pipelined by one iteration: MM2(i-1) is issued
            # between MM1-group(i) and the cast/transpose of iteration i,
            # so the in-order PE queue never parks waiting for the
            # ACT-cast + DVE-transpose round trip of its own iteration.
            def tail(pend):
                s1t_p, bp, hcp = pend
                # W-direction DCT, high-pass mask folded into wts2
                p2 = psum_pool.tile([P, W], F32, tag="p2", bufs=3)
                nc.tensor.matmul(p2[:], w2[:], s1t_p[:], start=True, stop=True)
                # block transpose back, straight out of PSUM
                s2t = work.tile([P, W], F32, tag="s2t", bufs=12)
                nc.vector.transpose(s2t[:], p2[:])
                # output DMA un-permutes the rows via the DRAM AP:
                # partition c2*32 + c1*16 + hb  ->  row hb*8 + c2*2 + c1
                dst = out[bp, 0, hcp * P:(hcp + 1) * P, :].rearrange(
                    "(hb c2 c1) w -> c2 c1 hb w", c2=4, c1=2
                )
                nc.sync.dma_start(dst, s2t[:])

            LAG = 3
            pending = []
            for b, hc in iters:
                # one 768 KB read: channels side by side in the free
                # dim, fp32 -> bf16 cast inline in the DMA (SWDGE)
                xt = xin.tile([P, C * W], BF16, tag="x")
                xsrc = x[b].rearrange("c (n p) w -> n p c w", p=P)[hc]
                nc.gpsimd.dma_start(
                    xt[:].rearrange("p (c w) -> p c w", w=W), xsrc
                )
                x0 = xt[:, 0 * W:1 * W]
                x1 = xt[:, 1 * W:2 * W]
                x2 = xt[:, 2 * W:3 * W]
                # grayscale (up to the w2 scale, folded into wts1), spread
                # over three engines so each stays under the input pace:
                # g0 = x0*ga + x2 (DVE), gs = x1*gb (ACT, 4x tensor-scalar
                # mode), g1 = g0 + gs (GpSimd)
                g0 = work.tile([P, W], BF16, tag="g0")
                nc.vector.scalar_tensor_tensor(g0[:], x0, ga, x2, mult, add)
                gs = work.tile([P, W], BF16, tag="gs")
                nc.scalar.mul(gs[:], x1, gb)
                g1 = work.tile([P, W], BF16, tag="g1")
                nc.gpsimd.tensor_tensor(g1[:], g0[:], gs[:], add)
                # H-direction DCT (permuted output rows)
                p1 = psum_pool.tile([P, W], F32, tag="p1")
                nc.tensor.matmul(p1[:], w1[:], g1[:], start=True, stop=True)
                # the W-DCT + transpose + store of the iteration LAG back:
                # MM2(i-LAG) precedes MM1(i+1) in the PE's in-order queue,
                # so the cast->transpose round trip of iteration i has LAG
                # iterations of slack before the PE needs its result --
                # the cross-engine latency does not set the iteration
                # period
                if len(pending) == LAG:
                    tail(pending.pop(0))
                # PSUM -> SBUF with fp32 -> bf16 cast on ACT, then the
                # 32x32 block transpose on DVE
                s1 = work.tile([P, W], BF16, tag="s1")
                nc.scalar.copy(s1[:], p1[:])
                s1t = work.tile([P, W], BF16, tag="s1t")
                nc.vector.transpose(s1t[:], s1[:])
                pending.append((s1t, b, hc))
            for pend in pending:
                tail(pend)
    nc.compile()
    return nc


def _host_constants(dct_matrix, mask):
    import ml_dtypes
    D = np.asarray(dct_matrix, dtype=np.float32)
    M = np.asarray(mask, dtype=np.float32)
    Wk = np.kron(np.eye(P // 8, dtype=np.float32), D.T).astype(np.float32)
    # MM1 stationary: w2-scaled (the grayscale ops compute gray/w2),
    # columns permuted to i' = u*16 + hb
    perm = np.array([(i % 16) * 8 + (i // 16) for i in range(P)])
    w1 = (np.float32(GRAY_W[2]) * Wk[:, perm]).astype(ml_dtypes.bfloat16)
    # MM2 stationary: high-pass mask folded in.  In the permuted layout,
    # output partition l carries (u = 2*(l//32) + {0,1}, v = l%8); the
    # masked (u<4 & v<4) region is exactly {l < 64 and l%8 < 4}.
    # Use the actual mask values so any non-binary mask still works.
    colmask = np.empty(P, dtype=np.float32)
    for l in range(P):
        u = 2 * (l // 32)          # mask[:cutoff] rows are constant per pair
        colmask[l] = M[u, l % 8]
    w2 = (Wk * colmask[None, :]).astype(ml_dtypes.bfloat16)
    return w1, w2


def kernel(x, dct_matrix, mask):
    global _NC, LAST_RUN
    x = np.ascontiguousarray(np.asarray(x, dtype=np.float32))
    assert x.shape == (B, C, H, W)
    w1, w2 = _host_constants(dct_matrix, mask)

    if _NC is None:
        _NC = _build_bass()

    in_maps = [
        {"x": np.ascontiguousarray(x[i * BLOC:(i + 1) * BLOC]),
         "wts1": w1[None], "wts2": w2[None]}
        for i in range(N_CORES)
    ]
    trace = bool(int(os.environ.get("DCT_TRACE", "0")))
    tdir = os.environ.get("DCT_TRACE_DIR")
    if tdir:
        os.makedirs(tdir, exist_ok=True)
    LAST_RUN = run_bass_kernel_spmd(
        _NC, in_maps, list(range(N_CORES)), trace=trace, tmpdir=tdir,
    )
    out = np.concatenate([LAST_RUN.results[i]["out"] for i in range(N_CORES)], axis=0)
    return out


# revision 34
# speedup vs baseline: 1.5723x; 1.5723x over previous
"""Trainium2 Bass kernel for blocked-DCT high-frequency extractor.

Computes, for x (64, 3, 512, 512) f32:
  gray = 0.299*R + 0.587*G + 0.114*B                     (B,1,H,W)
  per 8x8 block:  Y = mask * (D @ block @ D.T)           (2D DCT + high-pass)
  output (64, 1, 512, 512) f32

Strategy (pure data parallel over batch, 8 batches/core on 8 cores; the
kernel is HBM-bound: 24 MiB in + 8 MiB out per core).

Per core, per (batch, 128-row chunk) of the image:
  1. One 768 KB SWDGE DMA (gpsimd queue) brings all 3 channel chunks into
     a (128h, 3*512w) tile, CASTING fp32 -> bf16 in the DMA datapath.
  2. The grayscale weighted sum is folded into the H-direction DCT as a
     3-matmul PSUM accumulation: p1 = sum_c (w_c * W).T @ x_c where
     W = I_16 kron D^T.  No elementwise grayscale stage at all.
  3. ACT copies PSUM -> SBUF with fp32 -> bf16 cast (the ISA forbids
     casts inside StreamTranspose), then DVE does the 32x32 block
     transpose.  Because 8 | 32, this puts w%32 (which contains the
     intra-block w index) on partitions.
  4. W-direction DCT: one bf16 matmul with the same I_16 kron D^T.
  5. High-pass mask on ACT as two strided PSUM->SBUF copies: columns
     with u<4 are scaled by a per-partition 0/1 vector (zero iff v<4),
     u>=4 columns are a plain copy.
  6. DVE 32x32 block transpose back -> exact (h, w) output layout, fp32.
  7. 256 KB contiguous output DMA on the SP HWDGE queue (input owns the
     gpsimd SWDGE queue, so the two streams never share a ring).

The matmuls are bf16: the PE idles most of the kernel, so PE_HAM holds
it at the cold 1.2 GHz clock, and fp32 matmuls at 4x cycles were the
original bottleneck (114 us of TensorE busy).  All intermediate
precision is bf16, fine for the 2e-2 relative-error gate (measured
~5e-3).
"""

import os

import numpy as np

import concourse.bacc as bacc
import concourse.mybir as mybir
import concourse.tile as tile
from concourse.bass_utils import run_bass_kernel_spmd

N_CORES = 8
B, C, H, W = 64, 3, 512, 512
BLOC = B // N_CORES  # batches per core
P = 128              # SBUF partitions / chunk height
NCH = H // P         # 128-row chunks per image
F32 = mybir.dt.float32
BF16 = mybir.dt.bfloat16
GRAY_W = (0.299, 0.587, 0.114)

_NC = None          # cached compiled Bass module
LAST_RUN = None     # BassKernelResults of the most recent run (for test.py)


def _build_bass():
    nc = bacc.Bacc(
        "TRN2",
        target_bir_lowering=False,
        debug=False,
        num_devices=N_CORES,
    )
    x = nc.declare_dram_parameter("x", [BLOC, C, H, W], F32, isOutput=False)
    wts3 = nc.declare_dram_parameter("wts3", [C, P, P], BF16, isOutput=False)
    wts = nc.declare_dram_parameter("wts", [1, P, P], BF16, isOutput=False)
    mvec = nc.declare_dram_parameter("mvec", [P, 1], F32, isOutput=False)
    out = nc.declare_dram_parameter("out", [BLOC, 1, H, W], F32, isOutput=True)

    with tile.TileContext(nc) as tc:
        with (
            tc.tile_pool(name="consts", bufs=1) as consts,
            tc.tile_pool(name="xin", bufs=8) as xin,
            tc.tile_pool(name="work", bufs=6) as work,
            tc.tile_pool(name="psum", bufs=4, space="PSUM") as psum_pool,
        ):
            wd3 = consts.tile([P, C * P], BF16, tag="wd3")
            nc.sync.dma_start(
                wd3[:].rearrange("p (c q) -> p c q", q=P),
                wts3.rearrange("c p q -> p c q"),
            )
            wd = consts.tile([P, P], BF16, tag="wd")
            nc.sync.dma_start(wd[:], wts[0])
            mv = consts.tile([P, 1], F32, tag="mvec")
            nc.sync.dma_start(mv[:], mvec[:])

            for b in range(BLOC):
                for hc in range(NCH):
                    hs = hc * P
                    # one 768 KB read: channels side by side in the free
                    # dim, fp32 -> bf16 cast inline in the DMA (SWDGE)
                    xt = xin.tile([P, C * W], BF16, tag="x")
                    xsrc = x[b].rearrange("c (n p) w -> n p c w", p=P)[hc]
                    nc.gpsimd.dma_start(
                        xt[:].rearrange("p (c w) -> p c w", w=W), xsrc
                    )
                    # H-direction DCT with the grayscale sum folded in:
                    # p1 = sum_c (w_c * W).T @ x_c accumulated in PSUM
                    p1 = psum_pool.tile([P, W], F32, tag="p1")
                    for c in range(C):
                        nc.tensor.matmul(
                            p1[:], wd3[:, c * P:(c + 1) * P],
                            xt[:, c * W:(c + 1) * W],
                            start=(c == 0), stop=(c == C - 1),
                        )
                    # PSUM -> SBUF move with fp32 -> bf16 cast on ACT (the
                    # ISA forbids dtype casts inside StreamTranspose), then
                    # the 32x32 block transpose in bf16 on DVE
                    s1 = work.tile([P, W], BF16, tag="s1")
                    nc.scalar.copy(s1[:], p1[:])
                    s1t = work.tile([P, W], BF16, tag="s1t")
                    nc.vector.transpose(s1t[:], s1[:])
                    # W-direction DCT
                    p2 = psum_pool.tile([P, W], F32, tag="p2")
                    nc.tensor.matmul(p2[:], wd[:], s1t[:], start=True, stop=True)
                    # high-pass mask + PSUM->SBUF move on ACT: columns with
                    # u<4 get a per-partition 0/1 scale (zero iff v<4), the
                    # u>=4 columns are a plain copy.
                    s2 = work.tile([P, W], F32, tag="s2")
                    p2v = p2[:].rearrange("p (g u) -> p g u", u=8)
                    s2v = s2[:].rearrange("p (g u) -> p g u", u=8)
                    nc.scalar.mul(s2v[:, :, 0:4], p2v[:, :, 0:4], mv[:])
                    nc.scalar.copy(s2v[:, :, 4:8], p2v[:, :, 4:8])
                    # block transpose back to natural layout
                    s2t = work.tile([P, W], F32, tag="s2t", bufs=8)
                    nc.vector.transpose(s2t[:], s2[:])
                    # outputs ride the SP HWDGE queue; inputs own the
                    # gpsimd SWDGE queue
                    nc.sync.dma_start(out[b, 0, hs:hs + P, :], s2t[:])
    nc.compile()
    return nc


def _host_constants(dct_matrix, mask):
    import ml_dtypes
    D = np.asarray(dct_matrix, dtype=np.float32)
    M = np.asarray(mask, dtype=np.float32)
    Wk = np.kron(np.eye(P // 8, dtype=np.float32), D.T).astype(np.float32)
    # channel-scaled H-DCT weights (grayscale folded into the matmul)
    wts3 = np.stack([w * Wk for w in GRAY_W]).astype(ml_dtypes.bfloat16)
    wts = Wk[None].astype(ml_dtypes.bfloat16)
    # per-partition mask column for the u<4 output columns: M[u<4, v] is
    # constant in u there, so it reduces to a v-indexed 0/1 vector.
    pi = np.arange(P)
    mvec = np.ascontiguousarray(M[0, pi % 8], dtype=np.float32).reshape(P, 1)
    return wts3, wts, mvec


def kernel(x, dct_matrix, mask):
    global _NC, LAST_RUN
    x = np.ascontiguousarray(np.asarray(x, dtype=np.float32))
    assert x.shape == (B, C, H, W)
    wts3, wts, mvec = _host_constants(dct_matrix, mask)

    if _NC is None:
        _NC = _build_bass()

    in_maps = [
        {"x": np.ascontiguousarray(x[i * BLOC:(i + 1) * BLOC]),
         "wts3": wts3, "wts": wts, "mvec": mvec}
        for i in range(N_CORES)
    ]
    trace = bool(int(os.environ.get("DCT_TRACE", "0")))
    tdir = os.environ.get("DCT_TRACE_DIR")
    if tdir:
        os.makedirs(tdir, exist_ok=True)
    LAST_RUN = run_bass_kernel_spmd(
        _NC, in_maps, list(range(N_CORES)), trace=trace, tmpdir=tdir,
    )
    out = np.concatenate([LAST_RUN.results[i]["out"] for i in range(N_CORES)], axis=0)
    return out
